# revision 1
# baseline (speedup 1.0000x reference)
"""DNC-style LSTM-with-memory-read kernel for 8 Trainium2 NeuronCores.

Math summary (derived from the reference):
  The torch-faithful [R,B,M]->[B,R*M] view means row b' of the new read
  vector is concat_k read[(4*b'+k) mod B]. Since read = h @ mem_sm.T and
  rv only enters the LSTM through W_ih's rv columns (W_rv), the rv
  contribution to the gates collapses to a "mix" term:
      gates[b'] += sum_k h[4*u(b')+k] @ G_k,   u(b') = b' mod 256
  with G_k = mem_sm.T @ W_rv[:, k*M:(k+1)*M].T precomputed on host.
  The final fc layer is linear in h and read, and the output is a mean
  over time, so it reduces to a function of hsum = sum_t h_t — computed
  on host from each core's hsum shard.

Distribution: batch is sharded contiguously over 8 cores (128 rows each).
The mix term couples rows across shards (provably fully-mixing within 5
steps), so each step AllGathers hT (64KB/core). Each core then reads the
half of the gathered buffer its parity needs via a partition_id-derived
register offset, and computes its gates with 7 matmuls into one PSUM bank:
2 x-projection (prefetched), 1 W_hh, 4 mix (strided lhsT over gathered hT).
"""

import sys

if '/opt/trn_rl_repo' not in sys.path:
    sys.path.insert(0, '/opt/trn_rl_repo')

import numpy as np

B, T, D_IN = 1024, 128, 256
H = 128
M = 128
W = 128
R = 4
OUT = 2
NCORES = 8
RL = B // NCORES  # 128 local rows per core

_PROGRAM_CACHE = {}


def build_program(t_steps=T, bf16_mix=False, no_ag=False, ablate=()):
    """Build (and compile) the SPMD Bass program for t_steps timesteps.

    bf16_mix: carry the AllGathered hT and the mix matmuls in bf16.
    The mix term is ~4% of the gate magnitude, so bf16 there perturbs
    gates by ~2e-4 relative — negligible vs the fp32 recurrence.
    """
    import concourse.bass as bass
    import concourse.bacc as bacc
    import concourse.mybir as mybir
    import concourse.tile as tile
    from concourse.masks import make_identity

    f32 = mybir.dt.float32
    bf16 = mybir.dt.bfloat16
    mixdt = bf16 if bf16_mix else f32
    AF = mybir.ActivationFunctionType

    nc = bacc.Bacc(
        "TRN2",
        target_bir_lowering=False,
        debug=False,
        enable_asserts=False,
        num_devices=NCORES,
    )

    # Inputs (host-side layouts are pre-arranged for partition-major DMA)
    x_t_in = min(t_steps, T)
    xT = nc.dram_tensor("xT", [x_t_in, 128, 2, RL], f32, kind="ExternalInput")
    wxT = nc.dram_tensor("wxT", [128, 2, 512], f32, kind="ExternalInput")
    whhT = nc.dram_tensor("whhT", [128, 512], f32, kind="ExternalInput")
    gmat = nc.dram_tensor("gmat", [128, 4, 512], mixdt, kind="ExternalInput")
    biasb = nc.dram_tensor("biasb", [128, 512], f32, kind="ExternalInput")
    bias1b = nc.dram_tensor("bias1b", [128, 512], f32, kind="ExternalInput")
    hsum_out = nc.dram_tensor("hsum_out", [RL, H], f32, kind="ExternalOutput")

    with tile.TileContext(nc) as tc:
        with (
            tc.tile_pool(name="const", bufs=1) as cpool,
            tc.tile_pool(name="xin", bufs=4) as xpool,
            tc.tile_pool(name="work", bufs=2) as wpool,
            tc.tile_pool(name="gt", bufs=3) as gtpool,
            tc.tile_pool(name="psg", bufs=4, space="PSUM") as psg,
            tc.tile_pool(name="pst", bufs=2, space="PSUM") as pst,
            tc.tile_pool(name="dram", bufs=2, space="DRAM") as dpool,
        ):
            ident = cpool.tile([128, 128], f32)
            make_identity(nc, ident)
            wx_sb = cpool.tile([128, 2, 512], f32)
            nc.sync.dma_start(wx_sb[:], wxT[:])
            whh_sb = cpool.tile([128, 512], f32)
            nc.sync.dma_start(whh_sb[:], whhT[:])
            g_sb = cpool.tile([128, 4, 512], mixdt)
            nc.sync.dma_start(g_sb[:], gmat[:])
            bb_sb = cpool.tile([128, 512], f32)
            nc.sync.dma_start(bb_sb[:], biasb[:])
            b1_sb = cpool.tile([128, 512], f32)
            nc.sync.dma_start(b1_sb[:], bias1b[:])
            hsum = cpool.tile([RL, H], f32)
            nc.vector.memset(hsum[:], 0.0)

            # which half of the gathered hT this core's mix needs
            pid = nc.sync.partition_id()
            roff = nc.sync.compute_val((pid % 2) * 4)

            hT_sb = None
            hgat = None
            c_prev = None

            for t in range(1, t_steps + 1):
                # ---- x-projection for step t (independent of recurrence,
                #      prefetches ahead and fills PE gaps during AllGather)
                xt = xpool.tile([128, 2, RL], f32, tag="xt")
                nc.sync.dma_start(xt[:], xT[(t - 1) % x_t_in])
                psum_g = psg.tile([RL, 512], f32, tag="g")
                if "nope" not in ablate:
                    last_is_x = (t == 1) or ("nomix" in ablate and "nowhh" in ablate)
                    for c_ in range(2):
                        nc.tensor.matmul(
                            psum_g[:],
                            xt[:, c_, :],
                            wx_sb[:, c_, :],
                            start=(c_ == 0),
                            stop=(last_is_x and c_ == 1),
                        )
                    # ---- recurrent terms (use h_{t-1} local + gathered)
                    if t >= 2:
                        if "nowhh" not in ablate:
                            nc.tensor.matmul(
                                psum_g[:], hT_sb[:], whh_sb[:], start=False,
                                stop=("nomix" in ablate),
                            )
                        if "nomix" not in ablate:
                            if "nostride" in ablate:
                                for k in range(4):
                                    nc.tensor.matmul(
                                        psum_g[:],
                                        hgat[:, k, :],
                                        g_sb[:, k, :],
                                        start=False,
                                        stop=(k == 3),
                                    )
                            else:
                                hflat = hgat.rearrange("p r f -> p (r f)")
                                for k in range(4):
                                    nc.tensor.matmul(
                                        psum_g[:],
                                        hflat[:, k::4],
                                        g_sb[:, k, :],
                                        start=False,
                                        stop=(k == 3),
                                    )
                bias_t = b1_sb if t == 1 else bb_sb
                gates = gtpool.tile([RL, 512], f32, tag="gates")
                if "nope" in ablate:
                    nc.vector.tensor_copy(gates[:], bias_t[:])
                else:
                    for gi in range(4):
                        sl = slice(128 * gi, 128 * (gi + 1))
                        nc.vector.tensor_add(gates[:, sl], psum_g[:, sl], bias_t[:, sl])
                acti = wpool.tile([RL, 128], f32, tag="acti")
                actf = wpool.tile([RL, 128], f32, tag="actf")
                actg = wpool.tile([RL, 128], f32, tag="actg")
                acto = wpool.tile([RL, 128], f32, tag="acto")
                if "noact" in ablate:
                    nc.vector.tensor_copy(acti[:], gates[:, 0:128])
                    nc.vector.tensor_copy(actf[:], gates[:, 128:256])
                    nc.vector.tensor_copy(actg[:], gates[:, 256:384])
                    nc.vector.tensor_copy(acto[:], gates[:, 384:512])
                else:
                    nc.scalar.activation(acti[:], gates[:, 0:128], AF.Sigmoid)
                    nc.scalar.activation(actf[:], gates[:, 128:256], AF.Sigmoid)
                    nc.scalar.activation(actg[:], gates[:, 256:384], AF.Tanh)
                    nc.scalar.activation(acto[:], gates[:, 384:512], AF.Sigmoid)
                t2 = wpool.tile([RL, 128], f32, tag="t2")
                nc.vector.tensor_mul(t2[:], acti[:], actg[:])
                c_new = wpool.tile([RL, 128], f32, tag="c")
                if t == 1:
                    nc.vector.tensor_copy(c_new[:], t2[:])
                else:
                    t1 = wpool.tile([RL, 128], f32, tag="t1")
                    nc.vector.tensor_mul(t1[:], actf[:], c_prev[:])
                    nc.vector.tensor_add(c_new[:], t1[:], t2[:])
                c_prev = c_new
                tch = wpool.tile([RL, 128], f32, tag="tch")
                if "noact" in ablate:
                    nc.vector.tensor_copy(tch[:], c_new[:])
                else:
                    nc.scalar.activation(tch[:], c_new[:], AF.Tanh)
                h = wpool.tile([RL, 128], f32, tag="h")
                nc.vector.tensor_mul(h[:], acto[:], tch[:])
                nc.vector.tensor_add(hsum[:], hsum[:], h[:])
                if t < t_steps:
                    hT_sb = wpool.tile([128, RL], f32, tag="hT")
                    if "notr" in ablate or "nope" in ablate:
                        nc.vector.tensor_copy(hT_sb[:], h[:])
                    else:
                        ps_hT = pst.tile([128, RL], f32, tag="htr")
                        nc.tensor.transpose(ps_hT[:], h[:], ident[:])
                        nc.scalar.copy(hT_sb[:], ps_hT[:])
                    if bf16_mix:
                        hTb = wpool.tile([128, RL], bf16, tag="hTb")
                        nc.vector.tensor_copy(hTb[:], ps_hT[:])
                        ag_src = hTb
                    else:
                        ag_src = hT_sb
                    ag_in = dpool.tile([128, RL], mixdt, tag="agin")
                    nc.sync.dma_start(ag_in[:], ag_src[:])
                    if no_ag:
                        # timing-only variant: skip the collective, fake the
                        # gathered buffer from the local bounce (WRONG values)
                        hgat = gtpool.tile([128, 4, RL], mixdt, tag="hgat")
                        src = ag_in.rearrange("p f -> p f").broadcast_to(
                            [128, 4, RL]
                        ) if False else None
                        for r_ in range(4):
                            nc.sync.dma_start(hgat[:, r_, :], ag_in[:])
                    else:
                        ag_out = dpool.tile(
                            [NCORES * 128, RL], mixdt, tag="agout", addr_space="Shared"
                        )
                        nc.gpsimd.collective_compute(
                            "AllGather",
                            mybir.AluOpType.bypass,
                            replica_groups=[list(range(NCORES))],
                            ins=[ag_in[:]],
                            outs=[ag_out[:]],
                        )
                        hgat = gtpool.tile([128, 4, RL], mixdt, tag="hgat")
                        src = ag_out.rearrange("(r p) f -> p r f", p=128)[
                            :, bass.ds(roff, 4), :
                        ]
                        nc.sync.dma_start(hgat[:], src)

            nc.sync.dma_start(hsum_out[:], hsum[:])

    nc.compile()
    return nc


def build_program_r2(t_steps=T, t_block=16, fast=True):
    """R2: replicated full-batch recurrence, zero collectives in the loop.

    x is AllGathered once in t_block-sized chunks (hidden behind compute).
    Everything lives in transposed layout [units/features, rows]: h is
    produced directly as hT, ACT folds the per-unit bias, no transposes.
    Halves of the batch are processed sequentially to fit PSUM (4 gate
    banks + 2 mix banks + slack). Every core computes the full batch
    redundantly; output read from core 0.
    """
    import concourse.bass as bass
    import concourse.bacc as bacc
    import concourse.mybir as mybir
    import concourse.tile as tile

    f32 = mybir.dt.float32
    AF = mybir.ActivationFunctionType
    assert t_steps % t_block == 0
    n_blocks = t_steps // t_block
    x_t_in = min(t_steps, T)

    nc = bacc.Bacc(
        "TRN2",
        target_bir_lowering=False,
        debug=False,
        enable_asserts=False,
        num_devices=NCORES,
    )

    xT = nc.dram_tensor("xT", [x_t_in, 128, 2, RL], f32, kind="ExternalInput")
    wxT = nc.dram_tensor("wxT", [128, 2, 512], f32, kind="ExternalInput")
    whhT = nc.dram_tensor("whhT", [128, 512], f32, kind="ExternalInput")
    gmat = nc.dram_tensor("gmat", [128, 4, 512],
                          mybir.dt.bfloat16 if fast else f32,
                          kind="ExternalInput")
    biasc = nc.dram_tensor("biasc", [128, 4], f32, kind="ExternalInput")
    bias1c = nc.dram_tensor("bias1c", [128, 4], f32, kind="ExternalInput")
    hsum_out = nc.dram_tensor("hsum_out", [128, B], f32, kind="ExternalOutput")

    with tile.TileContext(nc) as tc:
        with (
            tc.tile_pool(name="const", bufs=1) as cpool,
            tc.tile_pool(name="xin", bufs=3) as xpool,
            tc.tile_pool(name="work", bufs=2) as wpool,
            tc.tile_pool(name="psg", bufs=5, space="PSUM") as psg,
            tc.tile_pool(name="psm", bufs=3, space="PSUM") as psm,
            tc.tile_pool(name="dram", bufs=2, space="DRAM") as dpool,
            tc.tile_pool(name="dramx", bufs=1, space="DRAM") as dxpool,
        ):
            wx_sb = cpool.tile([128, 2, 512], f32)
            nc.sync.dma_start(wx_sb[:], wxT[:])
            whh_sb = cpool.tile([128, 512], f32)
            nc.sync.dma_start(whh_sb[:], whhT[:])
            bf16 = mybir.dt.bfloat16
    
            g_sb = cpool.tile([128, 4, 512], bf16 if fast else f32)
            nc.sync.dma_start(g_sb[:], gmat[:])
            bb_sb = cpool.tile([128, 4], f32)
            nc.sync.dma_start(bb_sb[:], biasc[:])
            b1_sb = cpool.tile([128, 4], f32)
            nc.sync.dma_start(b1_sb[:], bias1c[:])
            hsum = cpool.tile([128, B], f32)
            nc.vector.memset(hsum[:], 0.0)

            # chunked AllGather of x (bounce own shard -> internal -> Shared)
            ag_blocks = []
            for bk in range(n_blocks):
                t0b = (bk * t_block) % x_t_in
                agx_in = dpool.tile([t_block * 128, 2 * RL], f32, tag="agxin")
                nc.sync.dma_start(
                    agx_in[:],
                    xT.ap()[t0b:t0b + t_block]
                    .rearrange("t p c f -> (t p) (c f)"),
                )
                agx_out = dxpool.tile(
                    [NCORES * t_block * 128, 2 * RL], f32, tag=f"agxout{bk}",
                    addr_space="Shared",
                )
                nc.gpsimd.collective_compute(
                    "AllGather",
                    mybir.AluOpType.bypass,
                    replica_groups=[list(range(NCORES))],
                    ins=[agx_in[:]],
                    outs=[agx_out[:]],
                )
                ag_blocks.append(agx_out)

            h_prev = None
            c_prev = None

            for t in range(1, t_steps + 1):
                bk, ti = (t - 1) // t_block, (t - 1) % t_block
                # xt: [128 p, 2 c, 1024 rows (r-major, f minor)]
                xt = xpool.tile([128, 2, NCORES, RL], f32, tag="xt")
                src = ag_blocks[bk].rearrange(
                    "(r t p) (c f) -> t p c r f", t=t_block, p=128, c=2
                )
                nc.sync.dma_start(xt[:], src[ti])

                if t >= 2:
                    # deinterleave h_prev cols into group-major [128, 4, 256]
                    dei = wpool.tile([128, 4, 256], bf16 if fast else f32,
                                     tag="dei")
                    nc.vector.tensor_copy(
                        dei[:], h_prev.rearrange("p (u k) -> p k u", k=4)
                    )
                    # mix psums: two banks, each [128, 512] packing 2 gates
                    mx = [psm.tile([128, 512], f32, tag="mx", name=f"mx{i_}")
                          for i_ in range(2)]
                    for g in range(4):
                        out_sl = mx[g // 2][:, 256 * (g % 2):256 * (g % 2 + 1)]
                        for k in range(4):
                            nc.tensor.matmul(
                                out_sl,
                                g_sb[:, k, 128 * g:128 * (g + 1)],
                                dei[:, k, :],
                                start=(k == 0),
                                stop=(k == 3),
                            )
                    # PSUM->SBUF: a DVE/ACT add may read at most one PSUM input
                    mxs = [wpool.tile([128, 512], f32, tag=f"mxs{i_}",
                                      name=f"mxs{i_}") for i_ in range(2)]
                    nc.scalar.copy(mxs[0][:], mx[0][:])
                    nc.scalar.copy(mxs[1][:], mx[1][:])

                bias_t = b1_sb if t == 1 else bb_sb
                acts = [wpool.tile([128, NCORES * RL], f32, tag=f"act{g}",
                                   name=f"act{g}")
                        for g in range(4)]
                pres = [wpool.tile([128, NCORES * RL], f32, tag=f"pre{g}",
                                   name=f"pre{g}")
                        for g in range(4)]
                for h_ in range(2):
                    rs = slice(512 * h_, 512 * (h_ + 1))
                    for g in range(4):
                        pg = psg.tile([128, 512], f32, tag="g")
                        for c_ in range(2):
                            nc.tensor.matmul(
                                pg[:],
                                wx_sb[:, c_, 128 * g:128 * (g + 1)],
                                xt[:, c_, 4 * h_:4 * (h_ + 1), :],
                                start=(c_ == 0),
                                stop=(t == 1 and c_ == 1),
                            )
                        fn_ = AF.Tanh if g == 2 else AF.Sigmoid
                        if t >= 2:
                            nc.tensor.matmul(
                                pg[:],
                                whh_sb[:, 128 * g:128 * (g + 1)],
                                h_prev[:, rs],
                                start=False,
                                stop=True,
                            )
                            mslice = mxs[g // 2][:, 256 * (g % 2):256 * (g % 2 + 1)]
                            rep = mslice.unsqueeze(1).broadcast_to([128, 2, 256])
                            # gates pre-activation to SBUF; one wide ACT/gate
                            nc.vector.tensor_add(
                                pres[g][:, rs].rearrange("p (a u) -> p a u", a=2),
                                pg.rearrange("p (a u) -> p a u", a=2),
                                rep,
                            )
                        else:
                            nc.scalar.activation(
                                acts[g][:, rs], pg[:], fn_,
                                bias=bias_t[:, g:g + 1]
                            )
                if t >= 2:
                    for g in range(4):
                        fn_ = AF.Tanh if g == 2 else AF.Sigmoid
                        nc.scalar.activation(
                            acts[g][:], pres[g][:], fn_, bias=bias_t[:, g:g + 1]
                        )

                t2 = wpool.tile([128, B], f32, tag="t2")
                nc.vector.tensor_mul(t2[:], acts[0][:], acts[2][:])
                c_new = wpool.tile([128, B], f32, tag="c")
                if t == 1:
                    nc.vector.tensor_copy(c_new[:], t2[:])
                else:
                    t1 = wpool.tile([128, B], f32, tag="t1")
                    nc.vector.tensor_mul(t1[:], acts[1][:], c_prev[:])
                    nc.vector.tensor_add(c_new[:], t1[:], t2[:])
                c_prev = c_new
                tch = wpool.tile([128, B], f32, tag="tch")
                nc.scalar.activation(tch[:], c_new[:], AF.Tanh)
                h_new = wpool.tile([128, B], f32, tag="h")
                nc.vector.tensor_mul(h_new[:], acts[3][:], tch[:])
                nc.vector.tensor_add(hsum[:], hsum[:], h_new[:])
                h_prev = h_new

            nc.sync.dma_start(hsum_out[:], hsum[:])

    nc.compile()
    return nc


def host_prep(inputs, t_steps=T, bf16_mix=False, mode="v1"):
    """Host-side parameter folding + per-core input maps."""
    x = np.asarray(inputs["x"], dtype=np.float32)
    memory = np.asarray(inputs["memory"], dtype=np.float64)
    rv0 = np.asarray(inputs["read_vectors0"], dtype=np.float64)
    W_ih = np.asarray(inputs["W_ih"], dtype=np.float64)
    W_hh = np.asarray(inputs["W_hh"], dtype=np.float64)
    b_ih = np.asarray(inputs["b_ih"], dtype=np.float64)
    b_hh = np.asarray(inputs["b_hh"], dtype=np.float64)

    # softmax over memory slots (dim 0)
    mm = memory - memory.max(axis=0, keepdims=True)
    e = np.exp(mm)
    mem_sm = e / e.sum(axis=0, keepdims=True)  # [M, W]

    W_x = W_ih[:, :D_IN]          # [4H, D_IN]
    W_rv = W_ih[:, D_IN:]         # [4H, R*W]
    bias = b_ih + b_hh            # [4H]
    bias1 = bias + rv0.reshape(R * W) @ W_rv.T

    # G[128k + j, c] = (mem_sm.T @ W_rv[:, kM:(k+1)M].T)[j, c]
    G = np.concatenate(
        [mem_sm.T @ W_rv[:, k * M:(k + 1) * M].T for k in range(R)], axis=0
    )  # [512, 4H]

    wxT_h = np.ascontiguousarray(
        W_x.T.reshape(2, 128, 4 * H).transpose(1, 0, 2), dtype=np.float32
    )
    whhT_h = np.ascontiguousarray(W_hh.T, dtype=np.float32)
    import ml_dtypes
    gdt = ml_dtypes.bfloat16 if bf16_mix else np.float32
    gmat_h = np.ascontiguousarray(
        G.reshape(4, 128, 4 * H).transpose(1, 0, 2).astype(gdt)
    )
    biasb_h = np.ascontiguousarray(
        np.broadcast_to(bias.astype(np.float32), (128, 4 * H))
    )
    bias1b_h = np.ascontiguousarray(
        np.broadcast_to(bias1.astype(np.float32), (128, 4 * H))
    )

    biasc_h = np.ascontiguousarray(
        bias.astype(np.float32).reshape(4, 128).T
    )
    bias1c_h = np.ascontiguousarray(
        bias1.astype(np.float32).reshape(4, 128).T
    )

    in_maps = []
    for d in range(NCORES):
        xs = x[d * RL:(d + 1) * RL, :t_steps, :]          # [RL, t, 256]
        x2 = xs.transpose(1, 2, 0)                        # [t, 256, RL]
        xT_h = np.ascontiguousarray(
            x2.reshape(t_steps, 2, 128, RL).transpose(0, 2, 1, 3)
        )                                                 # [t, 128, 2, RL]
        if mode == "r2":
            in_maps.append(
                {
                    "xT": xT_h,
                    "wxT": wxT_h,
                    "whhT": whhT_h,
                    "gmat": np.ascontiguousarray(
                        G.reshape(4, 128, 4 * H).transpose(1, 0, 2)
                        .astype(ml_dtypes.bfloat16)
                    ),
                    "biasc": biasc_h,
                    "bias1c": bias1c_h,
                }
            )
        else:
            in_maps.append(
                {
                    "xT": xT_h,
                    "wxT": wxT_h,
                    "whhT": whhT_h,
                    "gmat": gmat_h,
                    "biasb": biasb_h,
                    "bias1b": bias1b_h,
                }
            )
    return in_maps, mem_sm


def host_finish(inputs, hsum, t_steps=T):
    """Final fc layer + time-mean from hsum [B, H] (linear in hsum)."""
    memory = np.asarray(inputs["memory"], dtype=np.float64)
    fc_w = np.asarray(inputs["fc_w"], dtype=np.float64)
    fc_b = np.asarray(inputs["fc_b"], dtype=np.float64)

    mm = memory - memory.max(axis=0, keepdims=True)
    e = np.exp(mm)
    mem_sm = e / e.sum(axis=0, keepdims=True)

    fc_h = fc_w[:, :H]  # [OUT, H]
    Fstack = np.concatenate(
        [mem_sm.T @ fc_w[:, H + k * M:H + (k + 1) * M].T for k in range(R)],
        axis=0,
    )  # [512, OUT]

    hs = hsum.astype(np.float64)
    mixout = hs.reshape(B // 4, 4 * H) @ Fstack           # [256, OUT]
    out = (hs @ fc_h.T + mixout[np.arange(B) % (B // 4)]) / t_steps + fc_b
    return out.astype(np.float32)


BF16_MIX = False


def kernel(**inputs):
    """Entry point: full inputs in, full [B, OUT] output back.

    Uses the R2 program: replicated full-batch recurrence in transposed
    layout, x AllGathered once in chunks, zero collectives in the loop.
    """
    from concourse.bass_utils import run_bass_kernel_spmd

    key = ("r2", T)
    if key not in _PROGRAM_CACHE:
        _PROGRAM_CACHE[key] = build_program_r2(T, t_block=16)
    nc = _PROGRAM_CACHE[key]

    in_maps, _ = host_prep(inputs, T, mode="r2")
    res = run_bass_kernel_spmd(nc, in_maps, core_ids=list(range(NCORES)))
    hsumT = res.results[0]["hsum_out"]  # [128, B] (all cores identical)
    return host_finish(inputs, hsumT.T, T)

